# revision 30
# baseline (speedup 1.0000x reference)
"""Trainium2 Bass kernel: multi-head causal attention (B=2, T=2048, C=1024, H=16).

Sharding: 8 cores = data parallel over B (2) x tensor parallel over head
groups (4 groups of 4 heads).  Each core computes its batch's partial
output contribution from its 4 heads through Wo rows; the host sums the 4
partials per batch (the "all-reduce") and adds the folded biases
(bv @ Wo + bo).

v2 design — transposed-score attention, zero PE transposes:
  - Q/K/V are transposed AND pre-tiled on the HOST to [cb, t4, 128, 512]
    bf16 so every input DMA reads one contiguous 128KB block (1 DMA
    descriptor instead of 128 — the HW DGE is descriptor-rate-bound):
      qT/kT[d, t] = Wq^T X^T + b   (K=128 chunks over C, N=512 moving)
      v[t, d]     = (X^T chunk)^T @ Wv   (natural layout, attn@v stationary)
  - scores are computed TRANSPOSED per (head, 512-wide q chunk, 128-row
    k block):  scT[k, q] = kT_block^T @ qT_cols  — one K=64 matmul per
    k block, streaming only the causally-needed q columns.
  - exp on ACT (scale=1/8, f32 PSUM -> bf16 SBUF); additive -1e9 causal
    mask on the diagonal block (DVE).  Head-pipelined schedule: head h's
    scores/exp overlap head h-1's attn@v, so each exp has a full head
    loop of latency slack before the PE consumes it.
  - attn@v accumulates out_aug[65, q] = [v | 1]^T @ expT over k blocks in
    PSUM: rows 0-63 = unnormalized out^T, row 64 = softmax denominator S.
  - normalization: R = 1/S (DVE reciprocal of the S row), broadcast over
    the 64 dv lanes via a K=1 ones matmul (PE), one DVE multiply.
  - output projection fin[q, c] = outT_pair^T @ Wo rows (2 PSUM-accum
    matmuls); PSUM -> SBUF copy and the output DMA both issue on ACT so
    the store stream never FIFO-blocks input loads on the SP DMA queue.
    Output is pre-tiled [tb, cc, 128, 512] (1 descriptor per store);
    host combine un-tiles, sums the 4 group partials, adds the biases.
"""

from contextlib import ExitStack

import numpy as np
import ml_dtypes

import concourse.bass as bass
import concourse.mybir as mybir
import concourse.tile as tile
from concourse import bacc
from concourse.bass_utils import run_bass_kernel_spmd

B, T, C = 2, 2048, 1024
H, DK, DV = 16, 64, 64
N_CORES = 8
GROUPS = 4                 # head groups (tensor parallel)
HPG = H // GROUPS          # 4 heads per group
GD = HPG * DK              # 256 head dims per group
P = 128
TCH = 512                  # q/t chunk width (max moving free dim)
VW = DV + 1                # v columns per head incl. the ones column

BF = mybir.dt.bfloat16
F32 = mybir.dt.float32
F32R = mybir.dt.float32r
AX = mybir.AxisListType
AF = mybir.ActivationFunctionType

bf16 = ml_dtypes.bfloat16

# scheduling knobs; _NC_CACHE keys include these
CFG = {"sc_bufs": 3, "mm_bufs": 2, "aug_bufs": 3, "ex_bufs": 18,
       "fin_bufs": 4, "xin_bufs": 6, "fin_act": False,
       "norm": "full",   # "full" | "off" (timing probe only — wrong results)
       "phase": "all",   # "all" | "load" | "att" (timing probes)
       "av_lag": 2,      # lag sched: k-blocks attn@v trails the scores
       "sched": "hpipe"}  # "hpipe" (head-pipelined) | "lag"


def _emit_consts(nc, tc, io, ctx):
    """Constant/weight loads — hoisted OUT of the reps loop (weights stay
    resident in SBUF in steady state)."""
    NCB = C // P
    cpool = ctx.enter_context(tc.tile_pool(name="const", bufs=1))
    amask = cpool.tile([P, P], F32)   # [k, q] additive -1e9 where k > q
    nc.sync.dma_start(out=amask, in_=io["amaskt"][:, :])
    ones_sb = cpool.tile([P, DV], BF)  # row 64 used as K=1 bcast stationary
    nc.gpsimd.memset(ones_sb, 1.0)
    bq_sb = cpool.tile([P, 2], F32)
    nc.sync.dma_start(out=bq_sb, in_=io["bq"][:, :])
    bk_sb = cpool.tile([P, 2], F32)
    nc.sync.dma_start(out=bk_sb, in_=io["bk"][:, :])

    wq_sb = cpool.tile([P, NCB, GD], BF)
    wk_sb = cpool.tile([P, NCB, GD], BF)
    wv_sb = cpool.tile([P, NCB, GD], BF)
    for w_sb, name in ((wq_sb, "wq"), (wk_sb, "wk"), (wv_sb, "wv")):
        for cb in range(NCB):
            nc.sync.dma_start(out=w_sb[:, cb, :], in_=io[name][cb * P:(cb + 1) * P, :])
    wo_sb = cpool.tile([P, 2, C], BF)
    for pr in range(2):
        nc.sync.dma_start(out=wo_sb[:, pr, :], in_=io["wo"][pr * P:(pr + 1) * P, :])
    return amask, ones_sb, bq_sb, bk_sb, wq_sb, wk_sb, wv_sb, wo_sb


def _make_pools(tc, ctx):
    return {
        "spool": ctx.enter_context(tc.tile_pool(name="stream", bufs=2)),
        "ppool": ctx.enter_context(tc.tile_pool(name="pers", bufs=1)),
        "apool": ctx.enter_context(tc.tile_pool(name="attn", bufs=2)),
        "pp": ctx.enter_context(tc.tile_pool(name="ps", bufs=2, space="PSUM")),
    }


def _emit(nc, tc, io, t_len, ctx, consts, pools):
    NTC = t_len // TCH         # 512-wide chunks
    NT = t_len // P            # 128-row blocks
    NCB = C // P               # contraction chunks over C
    amask, ones_sb, bq_sb, bk_sb, wq_sb, wk_sb, wv_sb, wo_sb = consts

    spool, ppool = pools["spool"], pools["ppool"]
    apool, pp = pools["apool"], pools["pp"]

    # persistent activations
    qT_sb = ppool.tile([P, 2, t_len], BF)    # [pair head dims(128), pair, T]
    kT_sb = ppool.tile([P, 2, t_len], BF)
    v_sb = ppool.tile([P, NT, HPG * VW], BF)  # [T(k) blocks, (v(64)|1) x 4 heads]
    outT_sb = ppool.tile([P, 2, t_len], BF)   # [pair dv(128), pair, T]

    # ones columns of the augmented v (row 64 of out_aug = sum of exp = S)
    v_ones = v_sb.rearrange("p t (h e) -> p t h e", h=HPG)[:, :, :, DV:DV + 1]
    nc.gpsimd.memset(v_ones, 1.0)

    # ---- stage 1: projections for one 512-wide t chunk ----------------------
    def load_t4(t4):
        t0 = t4 * TCH
        for name, w_sb in (("qt", wq_sb), ("kt", wk_sb), ("vt", wv_sb)):
            xin = spool.tile([P, NCB, TCH], BF, tag="xin", bufs=CFG["xin_bufs"])
            # split input loads across both HW DMA queues: v on ACT's queue,
            # q/k on SP's (the load phase is DMA-queue-bound, not byte-bound)
            dma_eng = nc.scalar if name == "vt" else nc.sync
            for cb in range(NCB):
                dma_eng.dma_start(out=xin[:, cb, :], in_=io[name][cb, t4])
            if name == "vt":
                for tb in range(4):
                    ps = pp.tile([P, TCH], F32, tag="mm", bufs=CFG["mm_bufs"])
                    for cb in range(NCB):
                        nc.tensor.matmul(
                            ps[:, :GD], xin[:, cb, tb * P:(tb + 1) * P], w_sb[:, cb, :],
                            start=(cb == 0), stop=(cb == NCB - 1))
                    dst = v_sb[:, t4 * 4 + tb].rearrange(
                        "p (h e) -> p h e", h=HPG)[:, :, 0:DV]
                    src = ps[:, :GD].rearrange("p (h e) -> p h e", h=HPG)
                    nc.vector.tensor_copy(dst, src)
            else:
                xT = qT_sb if name == "qt" else kT_sb
                bias = bq_sb if name == "qt" else bk_sb
                for pr in range(2):
                    ps = pp.tile([P, TCH], F32, tag="mm", bufs=CFG["mm_bufs"])
                    for cb in range(NCB):
                        nc.tensor.matmul(
                            ps, w_sb[:, cb, pr * P:(pr + 1) * P], xin[:, cb, :],
                            start=(cb == 0), stop=(cb == NCB - 1))
                    nc.vector.tensor_scalar_add(
                        xT[:, pr, t0:t0 + TCH], ps, bias[:, pr:pr + 1])

    # ---- stage 2: attention for one (q chunk, head), scores transposed ------
    # normalize: R = 1/S; broadcast R across the 64 dv lanes with a K=1
    # ones matmul (f32r, full rate at N=512); one DVE scale into the slab
    def _norm(qc, h, aug):
        pr, hs = h // 2, (h % 2) * DK
        if CFG["norm"] == "off":   # timing probe: skip normalization entirely
            nc.vector.tensor_copy(
                outT_sb[0:DV, pr, qc * TCH:(qc + 1) * TCH], aug[0:DV, :])
            return
        Rrow = apool.tile([P, TCH], F32, tag="R", bufs=2)
        nc.vector.reciprocal(Rrow[DV:DV + 1, :], aug[DV:DV + 1, :])
        Rbf = apool.tile([P, TCH], BF, tag="Rb", bufs=2)
        nc.vector.tensor_copy(Rbf[DV:DV + 1, :], Rrow[DV:DV + 1, :])
        rbc = pp.tile([P, TCH], F32, tag="aug", bufs=CFG["aug_bufs"])
        nc.tensor.matmul(
            rbc[0:DV, :], ones_sb[DV:DV + 1, :],
            Rbf[DV:DV + 1, :], start=True, stop=True)
        rsc = apool.tile([P, TCH], F32, tag="rsc", bufs=2)
        nc.vector.tensor_copy(rsc[0:DV, :], rbc[0:DV, :])
        nc.vector.tensor_mul(
            outT_sb[hs:hs + DV, pr, qc * TCH:(qc + 1) * TCH],
            aug[0:DV, :], rsc[0:DV, :])

    def _score_exp(qc, h, kb):
        pr, hs = h // 2, (h % 2) * DK
        cols0 = max(0, kb - qc * 4) * P
        scp = pp.tile([P, TCH], F32, tag="sc", bufs=CFG["sc_bufs"])
        nc.tensor.matmul(
            scp[:, cols0:TCH],
            kT_sb[hs:hs + DK, pr, kb * P:(kb + 1) * P],
            qT_sb[hs:hs + DK, pr, qc * TCH + cols0:(qc + 1) * TCH],
            start=True, stop=True)
        if kb >= qc * 4:
            # additive causal mask on the diagonal 128-block
            j = kb - qc * 4
            nc.vector.tensor_add(
                scp[:, j * P:(j + 1) * P], scp[:, j * P:(j + 1) * P], amask)
        ex = apool.tile([P, TCH], BF, tag="ex", bufs=CFG["ex_bufs"])
        nc.scalar.activation(
            ex[:, cols0:TCH], scp[:, cols0:TCH], AF.Exp, scale=0.125)
        return (kb, cols0, ex)

    def attend_lag(qc):
        nkb = qc * 4 + 4
        pend_norm = None
        for h in range(HPG):
            aug = pp.tile([P, TCH], F32, tag="aug", bufs=CFG["aug_bufs"])
            pend = []
            for kb in range(nkb):
                pend.append(_score_exp(qc, h, kb))
                if len(pend) > CFG["av_lag"]:
                    _av(aug, h, nkb, *pend.pop(0))
                if pend_norm is not None and kb == 1:
                    # run the previous head's normalization once this head's
                    # PE pipeline is primed, so its matmul never stalls PE
                    _norm(*pend_norm)
                    pend_norm = None
            while pend:
                _av(aug, h, nkb, *pend.pop(0))
            pend_norm = (qc, h, aug)
        _norm(*pend_norm)

    def attend_hpipe(qc):
        # scores/exp of head h overlap attn@v of head h-1: every exp gets a
        # full head-loop of slack before PE consumes it
        nkb = qc * 4 + 4
        prev = None          # (h, aug, exs) whose attn@v runs this loop
        pend_norm = None
        for h in range(HPG):
            aug = pp.tile([P, TCH], F32, tag="aug", bufs=CFG["aug_bufs"])
            exs = []
            for kb in range(nkb):
                exs.append(_score_exp(qc, h, kb))
                if prev is not None:
                    _av(prev[1], prev[0], nkb, *prev[2][kb])
                if pend_norm is not None and kb == 1:
                    _norm(*pend_norm)
                    pend_norm = None
            if prev is not None:
                pend_norm = (qc, prev[0], prev[1])
            prev = (h, aug, exs)
        # epilogue: attn@v for the last head
        for kb in range(nkb):
            _av(prev[1], prev[0], nkb, *prev[2][kb])
            if pend_norm is not None and kb == 1:
                _norm(*pend_norm)
                pend_norm = None
        _norm(qc, prev[0], prev[1])

    def attend(qc):
        if CFG["sched"] == "hpipe":
            attend_hpipe(qc)
        else:
            attend_lag(qc)

    def _av(aug, h, nkb, kb, cols0, ex):
        nc.tensor.matmul(
            aug[0:VW, cols0:TCH], v_sb[:, kb, h * VW:(h + 1) * VW],
            ex[:, cols0:TCH], start=(kb == 0), stop=(kb == nkb - 1))

    # ---- stage 3: output projection for one q chunk (deferred one chunk) ----
    def outproj(qc):
        for tb in range(qc * 4, qc * 4 + 4):
            for cc in range(2):
                fp = pp.tile([P, TCH], F32, tag="sc", bufs=CFG["sc_bufs"])
                for pr in range(2):
                    nc.tensor.matmul(
                        fp, outT_sb[:, pr, tb * P:(tb + 1) * P],
                        wo_sb[:, pr, cc * TCH:(cc + 1) * TCH],
                        start=(pr == 0), stop=(pr == 1))
                fs = apool.tile([P, TCH], BF, tag="fin", bufs=CFG["fin_bufs"])
                # fin copy on DVE (slack there); the store issues on the ACT
                # DMA queue so it never FIFO-blocks input loads on SP
                nc.vector.tensor_copy(fs, fp)
                nc.scalar.dma_start(out=io["out"][tb, cc], in_=fs)

    # emission: attend(qc) right after load(qc); outproj deferred one chunk so
    # its PE matmuls never head-of-line block on the normalization DVE ops
    if CFG["phase"] == "load":     # timing probe: projections only
        for t4 in range(NTC):
            load_t4(t4)
        fs = apool.tile([P, TCH], BF, tag="fin", bufs=2)
        nc.vector.tensor_copy(fs[:, 0:HPG * VW], v_sb[:, NT - 1, :])
        nc.vector.tensor_copy(fs[:, 0:TCH], qT_sb[:, 1, t_len - TCH:])
        nc.vector.tensor_copy(fs[:, 0:TCH], kT_sb[:, 1, t_len - TCH:])
        nc.sync.dma_start(out=io["out"][0, 0], in_=fs)
    elif CFG["phase"] == "att":    # timing probe: no output projection
        load_t4(0)
        attend(0)
        for t4 in range(1, NTC):
            load_t4(t4)
            attend(t4)
        fs = apool.tile([P, TCH], BF, tag="fin", bufs=2)
        nc.vector.tensor_copy(fs, outT_sb[:, 0, 0:TCH])
        nc.sync.dma_start(out=io["out"][0, 0], in_=fs)
    else:
        load_t4(0)
        attend(0)
        for t4 in range(1, NTC):
            load_t4(t4)
            outproj(t4 - 1)
            attend(t4)
        outproj(NTC - 1)


def _build(t_len=T, reps=1):
    nc = bacc.Bacc("TRN2", target_bir_lowering=False, debug=False,
                   num_devices=N_CORES)
    io = {
        "qt": nc.dram_tensor("qt", [C // P, t_len // TCH, P, TCH], BF,
                             kind="ExternalInput"),
        "kt": nc.dram_tensor("kt", [C // P, t_len // TCH, P, TCH], BF,
                             kind="ExternalInput"),
        "vt": nc.dram_tensor("vt", [C // P, t_len // TCH, P, TCH], BF,
                             kind="ExternalInput"),
        "wq": nc.dram_tensor("wq", [C, GD], BF, kind="ExternalInput"),
        "wk": nc.dram_tensor("wk", [C, GD], BF, kind="ExternalInput"),
        "wv": nc.dram_tensor("wv", [C, GD], BF, kind="ExternalInput"),
        "wo": nc.dram_tensor("wo", [GD, C], BF, kind="ExternalInput"),
        "bq": nc.dram_tensor("bq", [P, 2], F32, kind="ExternalInput"),
        "bk": nc.dram_tensor("bk", [P, 2], F32, kind="ExternalInput"),
        "amaskt": nc.dram_tensor("amaskt", [P, P], F32, kind="ExternalInput"),
        "out": nc.dram_tensor("out", [t_len // P, C // TCH, P, TCH], BF,
                              kind="ExternalOutput"),
    }
    with tile.TileContext(nc) as tc, ExitStack() as ctx:
        consts = _emit_consts(nc, tc, io, ctx)
        pools = _make_pools(tc, ctx)
        if reps == 1:
            _emit(nc, tc, io, t_len, ctx, consts, pools)
        else:
            # unroll x2: the two body copies allocate independent tiles, so
            # iteration i+1's loads/projections overlap iteration i's tail
            # (no WAR serialization on the persistent qT/kT/v slabs)
            assert reps % 2 == 0, "reps must be even (x2-unrolled hw loop)"
            hints = (mybir.EngineType.PE, mybir.EngineType.DVE,
                     mybir.EngineType.Activation, mybir.EngineType.Pool,
                     mybir.EngineType.SP)
            with tc.For_i(0, reps // 2, 1, hint_engines=hints):
                _emit(nc, tc, io, t_len, ctx, consts, pools)
                _emit(nc, tc, io, t_len, ctx, consts, pools)
    nc.compile()
    return nc


_NC_CACHE = {}


def _get_nc(t_len=T, reps=1):
    key = (t_len, reps, tuple(sorted(CFG.items())))
    if key not in _NC_CACHE:
        _NC_CACHE[key] = _build(t_len, reps)
    return _NC_CACHE[key]


def make_in_maps(inputs, t_len=T):
    Q, K, V = inputs["Q"], inputs["K"], inputs["V"]
    Wq, bq = inputs["Wq"], inputs["bq"]
    Wk, bk = inputs["Wk"], inputs["bk"]
    Wv = inputs["Wv"]
    Wo = inputs["Wo"]
    amaskt = np.tril(np.full((P, P), -1e9, np.float32), -1)
    # host-side transposes + pre-tiling to [cb, t4, 128, 512] so every
    # device DMA reads one contiguous 128KB block (1 descriptor, not 128)
    def tilize(X):
        xt = np.asarray(X[:, :t_len]).reshape(B, t_len, C // P, P)
        # [b, t, cb, p] -> [b, cb, t4, p, tc]
        xt = xt.reshape(B, t_len // TCH, TCH, C // P, P)
        return np.ascontiguousarray(xt.transpose(0, 3, 1, 4, 2)).astype(bf16)

    QT, KT, VT = tilize(Q), tilize(K), tilize(V)
    in_maps = []
    for core in range(N_CORES):
        b, g = divmod(core, GROUPS)
        cs = slice(g * GD, (g + 1) * GD)
        in_maps.append({
            "qt": QT[b],
            "kt": KT[b],
            "vt": VT[b],
            "wq": np.ascontiguousarray(Wq[:, cs]).astype(bf16),
            "wk": np.ascontiguousarray(Wk[:, cs]).astype(bf16),
            "wv": np.ascontiguousarray(Wv[:, cs]).astype(bf16),
            "wo": np.ascontiguousarray(Wo[cs, :]).astype(bf16),
            "bq": np.ascontiguousarray(bq[cs].reshape(2, P).T).astype(np.float32),
            "bk": np.ascontiguousarray(bk[cs].reshape(2, P).T).astype(np.float32),
            "amaskt": amaskt,
        })
    return in_maps


def combine(results, inputs, t_len=T):
    bo, bv, Wo = inputs["bo"], inputs["bv"], inputs["Wo"]
    bias = (bo.astype(np.float64) + bv.astype(np.float64) @ Wo.astype(np.float64))
    out = np.empty((B, t_len, C), np.float32)
    for b in range(B):
        acc = np.zeros((t_len, C), np.float64)
        for g in range(GROUPS):
            o = results[b * GROUPS + g]["out"].astype(np.float64)
            acc += o.transpose(0, 2, 1, 3).reshape(t_len, C)
        out[b] = (acc + bias).astype(np.float32)
    return out


def _mask_is_causal(mask, t_len):
    mask = np.asarray(mask)
    if mask.shape != (1, 1, t_len, t_len):
        return False
    m = mask[0, 0]
    tri = np.tril(np.ones((t_len, t_len), bool))
    return (m[tri] == 0.0).all() and (m[~tri] <= -1e8).all()


def _reference_fallback(inputs):
    # generic-mask fallback (never hit with the causal reference mask)
    Q, K, V = (np.asarray(inputs[k], np.float32) for k in ("Q", "K", "V"))
    mask = np.asarray(inputs["mask"], np.float32)
    out = np.empty((B, T, C), np.float32)
    for b in range(B):
        acc = np.zeros((T, C), np.float32)
        for h in range(H):
            q = Q[b] @ inputs["Wq"][:, h * DK:(h + 1) * DK] + inputs["bq"][h * DK:(h + 1) * DK]
            k = K[b] @ inputs["Wk"][:, h * DK:(h + 1) * DK] + inputs["bk"][h * DK:(h + 1) * DK]
            v = V[b] @ inputs["Wv"][:, h * DV:(h + 1) * DV] + inputs["bv"][h * DV:(h + 1) * DV]
            m = mask[min(b, mask.shape[0] - 1), min(h, mask.shape[1] - 1)]
            s = (q @ k.T + m) / np.sqrt(DK).astype(np.float32)
            s -= s.max(-1, keepdims=True)
            e = np.exp(s)
            a = e / e.sum(-1, keepdims=True)
            acc += (a @ v) @ inputs["Wo"][h * DV:(h + 1) * DV, :]
        out[b] = acc + inputs["bo"]
    return out


def kernel(**inputs):
    inputs = {k: np.asarray(v) for k, v in inputs.items()}
    if not _mask_is_causal(inputs["mask"], T):
        return _reference_fallback(inputs)
    nc = _get_nc(T)
    in_maps = make_in_maps(inputs, T)
    res = run_bass_kernel_spmd(nc, in_maps, core_ids=list(range(N_CORES)))
    return combine(res.results, inputs, T)
